# revision 10
# baseline (speedup 1.0000x reference)
"""CharEmbeddingCNN Trainium2 kernel (fp8 one-hot table formulation).

Reference computation (per word of L=20 chars):
    xe = emb[x]                       # [L, 256] -> treated as [256, L]
    y_k = conv1d_valid(xe, w_k) + b_k # k in (3,4,5), 256 -> 256 channels
    out = relu(max over all (k, t) of y_k[:, t]) * (len != 0)

Because the "input" rows are one-hot selections from the embedding table,
the conv folds into per-tap alphabet tables:
    y_k[o, w, t] = b_k[o] + sum_dk A_{k,dk}[x[w, t+dk], o],
    A_{k,dk} = emb @ w_k[:, :, dk].T          # [256 alphabet, 256 out]

On device this is computed as one-hot(x) @ A matmuls with fp8e4 DoubleRow
perf mode: one matmul per tap contracts the WHOLE 256-char alphabet
(2 k-tiles of 128) at 1 column/cycle -- 2x the MAC rate of the bf16
conv formulation, and no embedding gathers (no GpSimd) at all.
One-hot activations are exact in fp8; only A is quantized (scaled by 2^9
into e4m3's normal range, rel err ~1e-2 on the final output, well under
the 2e-2 gate). The 2^-9 descale rides the existing relu*mask scale.

Strategy (data-parallel over 8 NeuronCores, 1024 words each):
  - Host packs one-hot chars as fp8 [128 part = ch%128, word, ch//128, pos]
    and tables as DoubleRow lhsT [128, ktile, (k,dk), out].
  - Groups of 28 words x lk positions (N = 504/476/448) per PSUM chain;
    k accumulating DoubleRow matmuls (one per tap) per chain.
  - Segment max over t via strided DVE reduce_max into per-k accumulators,
    incremental bias+max-combine across k, PE transpose to [word, ch], and
    a fused relu*(mask*2^-9) on ScalarE on the way out -- all overlapped
    with the matmul stream.
"""

import numpy as np
import ml_dtypes
from contextlib import ExitStack

import concourse.bacc as bacc
import concourse.tile as tile
from concourse import mybir
from concourse.bass_utils import run_bass_kernel_spmd

F32 = mybir.dt.float32
BF16 = mybir.dt.bfloat16
F8 = mybir.dt.float8e4
DRM = mybir.MatmulPerfMode.DoubleRow

B, S, L = 64, 128, 20
EMB = 256
KS = (3, 4, 5)
NCORES = 8
W = (B * S) // NCORES          # words per core (1024)
GW = 28                        # words per matmul group (N = GW * lk <= 504)
NKDK = sum(KS)                 # 12 packed (k, dk) table slices
SCALE = 2.0 ** 9               # fp8 table scale (into e4m3 normal range)
WARMUP_MM = 20                 # small matmuls to bridge until the first DMAs
                               # land; any PE idle gap drops the DVFS clock
                               # to 1.2GHz for ~4us, so bridge with no gap
DMA_CHUNK = 4                  # groups per oh DMA chunk
CB = 2                         # groups per combine region (small: short tail)


def _kdk_off(ki, dk):
    return sum(KS[:ki]) + dk


def build_bass(words=W):
    ngroups = (words + GW - 1) // GW
    nfull = words // 128        # full 128-word output blocks
    rpart = words % 128         # trailing partial output block

    nc = bacc.Bacc(
        "TRN2",
        target_bir_lowering=False,
        debug=False,
        enable_asserts=False,
        num_swdge_queues=1,
    )

    oh_d = nc.dram_tensor("oh", [128, words * 2 * L], F8,
                          kind="ExternalInput").ap()
    wt_d = nc.dram_tensor("wt8", [128, 2 * NKDK * EMB], F8,
                          kind="ExternalInput").ap()
    bias_d = nc.dram_tensor("bias", [128, 6], F32, kind="ExternalInput").ap()
    id_d = nc.dram_tensor("ident", [128, 128], F32, kind="ExternalInput").ap()
    out_d = nc.dram_tensor("out", [words, EMB], F32, kind="ExternalOutput").ap()

    with tile.TileContext(nc) as tc, ExitStack() as ctx:
        const_pool = ctx.enter_context(tc.tile_pool(name="const", bufs=1))
        psum_pool = ctx.enter_context(tc.tile_pool(name="ps", bufs=2, space="PSUM"))
        psum_t_pool = ctx.enter_context(tc.tile_pool(name="pst", bufs=2, space="PSUM"))
        m_pool = ctx.enter_context(tc.tile_pool(name="m", bufs=1))
        tmp_pool = ctx.enter_context(tc.tile_pool(name="tmp", bufs=2))
        out_pool = ctx.enter_context(tc.tile_pool(name="outp", bufs=3))

        # Input DMAs interleaved just-in-time: the k3 table slices and
        # first word-group land first, then k4/k5 slices between further
        # word chunks so no conv chain ever waits on the table DMA.
        wt_t = const_pool.tile([128, 2, NKDK, EMB], F8)
        wt_v = wt_d[:].rearrange("p (c f o) -> p c f o", c=2, f=NKDK)
        oh_t = const_pool.tile([128, words * 2 * L], F8)

        def oh_dma(w0, nw):
            nc.sync.dma_start(oh_t[:, w0 * 2 * L:(w0 + nw) * 2 * L],
                              oh_d[:, w0 * 2 * L:(w0 + nw) * 2 * L])

        nc.sync.dma_start(wt_t[:, :, 0:3, :], wt_v[:, :, 0:3, :])
        oh_dma(0, 32)
        nc.sync.dma_start(wt_t[:, :, 3:7, :], wt_v[:, :, 3:7, :])
        oh_dma(32, 32)
        nc.sync.dma_start(wt_t[:, :, 7:NKDK, :], wt_v[:, :, 7:NKDK, :])
        bias_t = const_pool.tile([128, 6], F32)
        nc.sync.dma_start(bias_t[:], bias_d[:])
        ident = const_pool.tile([128, 128], F32)
        nc.sync.dma_start(ident[:], id_d[:])
        oh_dma(64, 48)
        w0 = DMA_CHUNK * GW
        while w0 < words:
            nw = min(DMA_CHUNK * GW, words - w0)
            oh_dma(w0, nw)
            w0 += nw
        # [128, ktile, word, pos] view (strides: c=L, w=2L, t=1)
        oh_v = oh_t[:].rearrange("p (w c t) -> p c w t", c=2, t=L)

        M = {}
        for ki in range(3):
            for oc in range(2):
                M[(ki, oc)] = m_pool.tile(
                    [128, words], F32, tag=f"m{ki}{oc}", name=f"m{ki}{oc}")
        C = [m_pool.tile([128, words], F32, tag=f"c{oc}", name=f"c{oc}")
             for oc in range(2)]

        # PE warm-up on a memset scratch (no DMA dependency) while the
        # input DMAs land
        wscr = const_pool.tile([128, 256], BF16)
        nc.vector.memset(wscr[:], 0.0)
        warm = psum_pool.tile([128, 512], F32, tag="ps0")
        for _ in range(WARMUP_MM):
            nc.tensor.matmul(warm[:, :256], wscr[:, :128], wscr[:],
                             start=True, stop=True)

        wb_done = 0
        covered = 0

        def conv_group(gw0, gw, pst_ki=()):
            """One [gw-word x lk] rectangle: 3 convs x 2 o_chunks, PSUM-
            accumulated over taps with DoubleRow (full-alphabet) matmuls.
            Near the end of the stream the listed ki borrow the idle
            transpose banks so the PE never waits on the lagging DVE
            reduce drain (intra-rotation is covered by adjacent chains)."""
            for ki, k in enumerate(KS):
                for oc in range(2):
                    lk = L - k + 1
                    pool, tag = ((psum_t_pool, "pst") if ki in pst_ki
                                 else (psum_pool, f"ps{ki}"))
                    ps = pool.tile([128, gw, lk], F32, tag=tag, name=tag)
                    for dk in range(k):
                        nc.tensor.matmul(
                            ps[:],
                            wt_t[:, :, _kdk_off(ki, dk),
                                 oc * 128:(oc + 1) * 128],
                            oh_v[:, :, gw0:gw0 + gw, dk:dk + lk],
                            start=(dk == 0), stop=(dk == k - 1),
                            perf_mode=DRM,
                        )
                    nc.vector.reduce_max(
                        M[(ki, oc)][:, gw0:gw0 + gw], ps[:],
                        axis=mybir.AxisListType.X)

        def combine(hi):
            """Fold M into C for columns [covered, hi): bias adds on
            ScalarE, maxes on DVE."""
            nonlocal covered
            sl = slice(covered, hi)
            n = hi - covered
            for oc in range(2):
                t4 = tmp_pool.tile([128, n], F32, tag="t4", name="t4")
                nc.scalar.add(
                    C[oc][:, sl], M[(0, oc)][:, sl],
                    bias_t[:, 3 * oc:3 * oc + 1])
                nc.scalar.add(
                    t4[:], M[(1, oc)][:, sl], bias_t[:, 3 * oc + 1:3 * oc + 2])
                nc.vector.tensor_max(C[oc][:, sl], C[oc][:, sl], t4[:])
                nc.scalar.add(
                    t4[:], M[(2, oc)][:, sl], bias_t[:, 3 * oc + 2:3 * oc + 3])
                nc.vector.tensor_max(C[oc][:, sl], C[oc][:, sl], t4[:])
            covered = hi

        def emit_ready():
            """Emit finished 128-word output blocks (transpose + relu*mask,
            where mask bakes in the 2^-9 descale)."""
            nonlocal wb_done
            while (wb_done + 1) * 128 <= covered:
                wb = wb_done
                for oc in range(2):
                    pst = psum_t_pool.tile([128, 128], F32, tag="pst",
                                           name="pst")
                    nc.tensor.transpose(
                        pst[:], C[oc][:, wb * 128:(wb + 1) * 128], ident[:])
                    ot = out_pool.tile([128, 128], F32, tag="ot", name="ot")
                    nc.scalar.activation(
                        ot[:], pst[:], mybir.ActivationFunctionType.Relu,
                        scale=1.0 / SCALE)
                    nc.sync.dma_start(
                        out_d[wb * 128:(wb + 1) * 128,
                              oc * 128:(oc + 1) * 128], ot[:])
                wb_done += 1

        # Remainder group last: the final serialized reduce+combine tail
        # covers only 16 words. Combine per-group near the end so the last
        # combine is minimal.
        groups = [(j * GW, GW) for j in range(ngroups - 1)]
        groups.append(((ngroups - 1) * GW, words - (ngroups - 1) * GW))
        for g, (gw0, gw) in enumerate(groups):
            emit_ready()
            pk = ((0, 1, 2) if g == len(groups) - 1
                  else (2,) if g >= len(groups) - 3 else ())
            conv_group(gw0, gw, pst_ki=pk)
            if g % CB == CB - 1 or g >= len(groups) - 3:
                combine(gw0 + gw)
        # Hold the PE clock through the serial reduce/combine tail so the
        # final transposes+relus run at full speed (ps0 banks are several
        # groups cold here, so these issue without waits).
        for _ in range(8):
            nc.tensor.matmul(warm[:, :128], wscr[:, :128], wscr[:, :128],
                             start=True, stop=True)
        emit_ready()
        assert covered == words and wb_done == nfull
        if rpart:
            for oc in range(2):
                pst = psum_t_pool.tile([128, 128], F32, tag="pst",
                                       name="pst")
                nc.tensor.transpose(
                    pst[0:rpart, :], C[oc][:, nfull * 128:words], ident[:])
                ot = out_pool.tile([128, 128], F32, tag="ot", name="ot")
                nc.scalar.activation(
                    ot[0:rpart, :], pst[0:rpart, :],
                    mybir.ActivationFunctionType.Relu, scale=1.0 / SCALE)
                nc.sync.dma_start(
                    out_d[nfull * 128:words, oc * 128:(oc + 1) * 128],
                    ot[0:rpart, :])

    nc.compile()
    return nc


def prep_shared(emb, w3, w4, w5, b3, b4, b5):
    """fp8 DoubleRow lhsT tables wt8[p, ktile, (k,dk), o], scaled bias."""
    emb64 = emb.astype(np.float64)
    wta = np.empty((EMB, NKDK, EMB), dtype=np.float64)
    for ki, w in enumerate((w3, w4, w5)):
        for dk in range(KS[ki]):
            # wta[c, off, o] = sum_i emb[c, i] w[o, i, dk]
            wta[:, _kdk_off(ki, dk), :] = emb64 @ w[:, :, dk].astype(np.float64).T
    wt8 = (wta * SCALE).reshape(2, 128, NKDK, EMB).transpose(1, 0, 2, 3)
    wt8 = np.ascontiguousarray(wt8.astype(ml_dtypes.float8_e4m3)).reshape(128, -1)
    bias = np.empty((128, 6), dtype=np.float32)
    for oc in range(2):
        for ki, b in enumerate((b3, b4, b5)):
            bias[:, 3 * oc + ki] = b[oc * 128:(oc + 1) * 128] * SCALE
    ident = np.eye(128, dtype=np.float32)
    return wt8, bias, ident


def prep_core(xf):
    """Per-core one-hot packing. xf: [words, L] int32.
    oh[p, (w, c, t)] = (xf[w, t] == c*128 + p), fp8."""
    words = xf.shape[0]
    n = words * L
    oh = np.zeros((n, EMB), dtype=np.uint8)
    oh[np.arange(n), xf.reshape(-1)] = 1
    oh = (oh.reshape(words, L, 2, 128).transpose(3, 0, 2, 1)
          .astype(ml_dtypes.float8_e4m3).reshape(128, -1))
    return np.ascontiguousarray(oh)


_CACHE = {}


def _get_nc(words=W):
    if words not in _CACHE:
        _CACHE[words] = build_bass(words)
    return _CACHE[words]


def run(x, lens, emb, w3, b3, w4, b4, w5, b5, trace=False, **spmd_kwargs):
    """Words with len == 0 are masked to zero by the reference, so the host
    compacts the nonzero-len words across all cores (~4.7% fewer rows on
    device) and scatters the device outputs back into a zero canvas."""
    x = np.asarray(x)
    lens = np.asarray(lens)
    wt8, bias, ident = prep_shared(
        np.asarray(emb, dtype=np.float32), np.asarray(w3), np.asarray(w4),
        np.asarray(w5), np.asarray(b3), np.asarray(b4), np.asarray(b5))
    xf = x.reshape(B * S, L)
    lensf = lens.reshape(B * S)
    nz = np.nonzero(lensf)[0]
    full = np.zeros((B * S, EMB), dtype=np.float32)
    if len(nz):
        wpc = -(-len(nz) // NCORES)
        idx = np.concatenate(
            [nz, np.full(wpc * NCORES - len(nz), nz[0], dtype=nz.dtype)])
        nc = _get_nc(wpc)
        in_maps = []
        for c in range(NCORES):
            oh = prep_core(xf[idx[c * wpc:(c + 1) * wpc]])
            in_maps.append({
                "oh": oh, "wt8": wt8, "bias": bias, "ident": ident,
            })
        res = run_bass_kernel_spmd(
            nc, in_maps, core_ids=list(range(NCORES)), trace=trace,
            **spmd_kwargs)
        out = np.concatenate([r["out"] for r in res.results], axis=0)
        full[nz] = out[:len(nz)]
    else:
        res = None
    return np.ascontiguousarray(full.reshape(B, S, EMB)), res


def kernel(x, lens, emb, w3, b3, w4, b4, w5, b5, **unused):
    out, _ = run(x, lens, emb, w3, b3, w4, b4, w5, b5)
    return out


# revision 11
# speedup vs baseline: 1.0039x; 1.0039x over previous
"""CharEmbeddingCNN Trainium2 kernel (fp8 one-hot table formulation).

Reference computation (per word of L=20 chars):
    xe = emb[x]                       # [L, 256] -> treated as [256, L]
    y_k = conv1d_valid(xe, w_k) + b_k # k in (3,4,5), 256 -> 256 channels
    out = relu(max over all (k, t) of y_k[:, t]) * (len != 0)

Because the "input" rows are one-hot selections from the embedding table,
the conv folds into per-tap alphabet tables:
    y_k[o, w, t] = b_k[o] + sum_dk A_{k,dk}[x[w, t+dk], o],
    A_{k,dk} = emb @ w_k[:, :, dk].T          # [256 alphabet, 256 out]

On device this is computed as one-hot(x) @ A matmuls with fp8e4 DoubleRow
perf mode: one matmul per tap contracts the WHOLE 256-char alphabet
(2 k-tiles of 128) at 1 column/cycle -- 2x the MAC rate of the bf16
conv formulation, and no embedding gathers (no GpSimd) at all.
One-hot activations are exact in fp8; only A is quantized (scaled by 2^9
into e4m3's normal range, rel err ~1e-2 on the final output, well under
the 2e-2 gate). The 2^-9 descale rides the existing relu*mask scale.

Strategy (data-parallel over 8 NeuronCores, 1024 words each):
  - Host packs one-hot chars as fp8 [128 part = ch%128, word, ch//128, pos]
    and tables as DoubleRow lhsT [128, ktile, (k,dk), out].
  - Groups of 28 words x lk positions (N = 504/476/448) per PSUM chain;
    k accumulating DoubleRow matmuls (one per tap) per chain.
  - Segment max over t via strided DVE reduce_max into per-k accumulators,
    incremental bias+max-combine across k, PE transpose to [word, ch], and
    a fused relu*(mask*2^-9) on ScalarE on the way out -- all overlapped
    with the matmul stream.
"""

import numpy as np
import ml_dtypes
from contextlib import ExitStack

import concourse.bacc as bacc
import concourse.tile as tile
from concourse import mybir
from concourse.bass_utils import run_bass_kernel_spmd

F32 = mybir.dt.float32
BF16 = mybir.dt.bfloat16
F8 = mybir.dt.float8e4
DRM = mybir.MatmulPerfMode.DoubleRow

B, S, L = 64, 128, 20
EMB = 256
KS = (3, 4, 5)
NCORES = 8
W = (B * S) // NCORES          # words per core (1024)
GW = 28                        # words per matmul group (N = GW * lk <= 504)
NKDK = sum(KS)                 # 12 packed (k, dk) table slices
SCALE = 2.0 ** 9               # fp8 table scale (into e4m3 normal range)
WARMUP_MM = 20                 # small matmuls to bridge until the first DMAs
                               # land; any PE idle gap drops the DVFS clock
                               # to 1.2GHz for ~4us, so bridge with no gap
DMA_CHUNK = 4                  # groups per oh DMA chunk
CB = 2                         # groups per combine region (small: short tail)


def _kdk_off(ki, dk):
    return sum(KS[:ki]) + dk


def build_bass(words=W):
    ngroups = (words + GW - 1) // GW
    nfull = words // 128        # full 128-word output blocks
    rpart = words % 128         # trailing partial output block

    nc = bacc.Bacc(
        "TRN2",
        target_bir_lowering=False,
        debug=False,
        enable_asserts=False,
        num_swdge_queues=1,
    )

    oh_d = nc.dram_tensor("oh", [128, words * 2 * L], F8,
                          kind="ExternalInput").ap()
    wt_d = nc.dram_tensor("wt8", [128, 2 * NKDK * EMB], F8,
                          kind="ExternalInput").ap()
    bias_d = nc.dram_tensor("bias", [128, 6], F32, kind="ExternalInput").ap()
    id_d = nc.dram_tensor("ident", [128, 128], F32, kind="ExternalInput").ap()
    out_d = nc.dram_tensor("out", [words, EMB], F32, kind="ExternalOutput").ap()

    with tile.TileContext(nc) as tc, ExitStack() as ctx:
        const_pool = ctx.enter_context(tc.tile_pool(name="const", bufs=1))
        psum_pool = ctx.enter_context(tc.tile_pool(name="ps", bufs=2, space="PSUM"))
        psum_t_pool = ctx.enter_context(tc.tile_pool(name="pst", bufs=2, space="PSUM"))
        m_pool = ctx.enter_context(tc.tile_pool(name="m", bufs=1))
        tmp_pool = ctx.enter_context(tc.tile_pool(name="tmp", bufs=2))
        out_pool = ctx.enter_context(tc.tile_pool(name="outp", bufs=3))

        # Input DMAs interleaved just-in-time: the k3 table slices and
        # first word-group land first, then k4/k5 slices between further
        # word chunks so no conv chain ever waits on the table DMA.
        wt_t = const_pool.tile([128, 2, NKDK, EMB], F8)
        wt_v = wt_d[:].rearrange("p (c f o) -> p c f o", c=2, f=NKDK)
        oh_t = const_pool.tile([128, words * 2 * L], F8)

        def oh_dma(w0, nw):
            nc.sync.dma_start(oh_t[:, w0 * 2 * L:(w0 + nw) * 2 * L],
                              oh_d[:, w0 * 2 * L:(w0 + nw) * 2 * L])

        nc.sync.dma_start(wt_t[:, :, 0:3, :], wt_v[:, :, 0:3, :])
        oh_dma(0, 32)
        nc.sync.dma_start(wt_t[:, :, 3:7, :], wt_v[:, :, 3:7, :])
        oh_dma(32, 32)
        nc.sync.dma_start(wt_t[:, :, 7:NKDK, :], wt_v[:, :, 7:NKDK, :])
        bias_t = const_pool.tile([128, 6], F32)
        nc.sync.dma_start(bias_t[:], bias_d[:])
        ident = const_pool.tile([128, 128], F32)
        nc.sync.dma_start(ident[:], id_d[:])
        oh_dma(64, 48)
        w0 = DMA_CHUNK * GW
        while w0 < words:
            nw = min(DMA_CHUNK * GW, words - w0)
            oh_dma(w0, nw)
            w0 += nw
        # [128, ktile, word, pos] view (strides: c=L, w=2L, t=1)
        oh_v = oh_t[:].rearrange("p (w c t) -> p c w t", c=2, t=L)

        M = {}
        for ki in range(3):
            for oc in range(2):
                M[(ki, oc)] = m_pool.tile(
                    [128, words], F32, tag=f"m{ki}{oc}", name=f"m{ki}{oc}")
        C = [m_pool.tile([128, words], F32, tag=f"c{oc}", name=f"c{oc}")
             for oc in range(2)]

        # PE warm-up on a memset scratch (no DMA dependency) while the
        # input DMAs land
        wscr = const_pool.tile([128, 256], BF16)
        nc.vector.memset(wscr[:], 0.0)
        warm = psum_pool.tile([128, 512], F32, tag="ps0")
        for _ in range(WARMUP_MM):
            nc.tensor.matmul(warm[:, :256], wscr[:, :128], wscr[:],
                             start=True, stop=True)

        wb_done = 0
        covered = 0

        def conv_group(gw0, gw, pst_ki=()):
            """One [gw-word x lk] rectangle: 3 convs x 2 o_chunks, PSUM-
            accumulated over taps with DoubleRow (full-alphabet) matmuls.
            Near the end of the stream the listed ki borrow the idle
            transpose banks so the PE never waits on the lagging DVE
            reduce drain (intra-rotation is covered by adjacent chains)."""
            for ki, k in enumerate(KS):
                for oc in range(2):
                    lk = L - k + 1
                    pool, tag = ((psum_t_pool, "pst") if ki in pst_ki
                                 else (psum_pool, f"ps{ki}"))
                    ps = pool.tile([128, gw, lk], F32, tag=tag, name=tag)
                    for dk in range(k):
                        nc.tensor.matmul(
                            ps[:],
                            wt_t[:, :, _kdk_off(ki, dk),
                                 oc * 128:(oc + 1) * 128],
                            oh_v[:, :, gw0:gw0 + gw, dk:dk + lk],
                            start=(dk == 0), stop=(dk == k - 1),
                            perf_mode=DRM,
                        )
                    nc.vector.reduce_max(
                        M[(ki, oc)][:, gw0:gw0 + gw], ps[:],
                        axis=mybir.AxisListType.X)

        def combine(hi):
            """Fold M into C for columns [covered, hi): bias adds on
            ScalarE, maxes on DVE."""
            nonlocal covered
            sl = slice(covered, hi)
            n = hi - covered
            for oc in range(2):
                t4 = tmp_pool.tile([128, n], F32, tag="t4", name="t4")
                nc.scalar.add(
                    C[oc][:, sl], M[(0, oc)][:, sl],
                    bias_t[:, 3 * oc:3 * oc + 1])
                nc.scalar.add(
                    t4[:], M[(1, oc)][:, sl], bias_t[:, 3 * oc + 1:3 * oc + 2])
                nc.vector.tensor_max(C[oc][:, sl], C[oc][:, sl], t4[:])
                nc.scalar.add(
                    t4[:], M[(2, oc)][:, sl], bias_t[:, 3 * oc + 2:3 * oc + 3])
                nc.vector.tensor_max(C[oc][:, sl], C[oc][:, sl], t4[:])
            covered = hi

        def emit_ready():
            """Emit finished 128-word output blocks (transpose + relu*mask,
            where mask bakes in the 2^-9 descale)."""
            nonlocal wb_done
            while (wb_done + 1) * 128 <= covered:
                wb = wb_done
                for oc in range(2):
                    pst = psum_t_pool.tile([128, 128], F32, tag="pst",
                                           name="pst")
                    nc.tensor.transpose(
                        pst[:], C[oc][:, wb * 128:(wb + 1) * 128], ident[:])
                    ot = out_pool.tile([128, 128], F32, tag="ot", name="ot")
                    nc.scalar.activation(
                        ot[:], pst[:], mybir.ActivationFunctionType.Relu,
                        scale=1.0 / SCALE)
                    nc.sync.dma_start(
                        out_d[wb * 128:(wb + 1) * 128,
                              oc * 128:(oc + 1) * 128], ot[:])
                wb_done += 1

        # Remainder group last: the final serialized reduce+combine tail
        # covers only 16 words. Combine per-group near the end so the last
        # combine is minimal.
        groups = [(j * GW, GW) for j in range(ngroups - 1)]
        groups.append(((ngroups - 1) * GW, words - (ngroups - 1) * GW))
        for g, (gw0, gw) in enumerate(groups):
            emit_ready()
            pk = ((0, 1, 2) if g == len(groups) - 1
                  else (2,) if g >= len(groups) - 3 else ())
            conv_group(gw0, gw, pst_ki=pk)
            if g % CB == CB - 1 or g >= len(groups) - 3:
                combine(gw0 + gw)
        emit_ready()
        assert covered == words and wb_done == nfull
        if rpart:
            for oc in range(2):
                pst = psum_t_pool.tile([128, 128], F32, tag="pst",
                                       name="pst")
                nc.tensor.transpose(
                    pst[0:rpart, :], C[oc][:, nfull * 128:words], ident[:])
                ot = out_pool.tile([128, 128], F32, tag="ot", name="ot")
                nc.scalar.activation(
                    ot[0:rpart, :], pst[0:rpart, :],
                    mybir.ActivationFunctionType.Relu, scale=1.0 / SCALE)
                nc.sync.dma_start(
                    out_d[nfull * 128:words, oc * 128:(oc + 1) * 128],
                    ot[0:rpart, :])

    nc.compile()
    return nc


def prep_shared(emb, w3, w4, w5, b3, b4, b5):
    """fp8 DoubleRow lhsT tables wt8[p, ktile, (k,dk), o], scaled bias."""
    emb64 = emb.astype(np.float64)
    wta = np.empty((EMB, NKDK, EMB), dtype=np.float64)
    for ki, w in enumerate((w3, w4, w5)):
        for dk in range(KS[ki]):
            # wta[c, off, o] = sum_i emb[c, i] w[o, i, dk]
            wta[:, _kdk_off(ki, dk), :] = emb64 @ w[:, :, dk].astype(np.float64).T
    wt8 = (wta * SCALE).reshape(2, 128, NKDK, EMB).transpose(1, 0, 2, 3)
    wt8 = np.ascontiguousarray(wt8.astype(ml_dtypes.float8_e4m3)).reshape(128, -1)
    bias = np.empty((128, 6), dtype=np.float32)
    for oc in range(2):
        for ki, b in enumerate((b3, b4, b5)):
            bias[:, 3 * oc + ki] = b[oc * 128:(oc + 1) * 128] * SCALE
    ident = np.eye(128, dtype=np.float32)
    return wt8, bias, ident


def prep_core(xf):
    """Per-core one-hot packing. xf: [words, L] int32.
    oh[p, (w, c, t)] = (xf[w, t] == c*128 + p), fp8."""
    words = xf.shape[0]
    n = words * L
    oh = np.zeros((n, EMB), dtype=np.uint8)
    oh[np.arange(n), xf.reshape(-1)] = 1
    oh = (oh.reshape(words, L, 2, 128).transpose(3, 0, 2, 1)
          .astype(ml_dtypes.float8_e4m3).reshape(128, -1))
    return np.ascontiguousarray(oh)


_CACHE = {}


def _get_nc(words=W):
    if words not in _CACHE:
        _CACHE[words] = build_bass(words)
    return _CACHE[words]


def run(x, lens, emb, w3, b3, w4, b4, w5, b5, trace=False, **spmd_kwargs):
    """Words with len == 0 are masked to zero by the reference, so the host
    compacts the nonzero-len words across all cores (~4.7% fewer rows on
    device) and scatters the device outputs back into a zero canvas."""
    x = np.asarray(x)
    lens = np.asarray(lens)
    wt8, bias, ident = prep_shared(
        np.asarray(emb, dtype=np.float32), np.asarray(w3), np.asarray(w4),
        np.asarray(w5), np.asarray(b3), np.asarray(b4), np.asarray(b5))
    xf = x.reshape(B * S, L)
    lensf = lens.reshape(B * S)
    nz = np.nonzero(lensf)[0]
    full = np.zeros((B * S, EMB), dtype=np.float32)
    if len(nz):
        wpc = -(-len(nz) // NCORES)
        idx = np.concatenate(
            [nz, np.full(wpc * NCORES - len(nz), nz[0], dtype=nz.dtype)])
        nc = _get_nc(wpc)
        in_maps = []
        for c in range(NCORES):
            oh = prep_core(xf[idx[c * wpc:(c + 1) * wpc]])
            in_maps.append({
                "oh": oh, "wt8": wt8, "bias": bias, "ident": ident,
            })
        res = run_bass_kernel_spmd(
            nc, in_maps, core_ids=list(range(NCORES)), trace=trace,
            **spmd_kwargs)
        out = np.concatenate([r["out"] for r in res.results], axis=0)
        full[nz] = out[:len(nz)]
    else:
        res = None
    return np.ascontiguousarray(full.reshape(B, S, EMB)), res


def kernel(x, lens, emb, w3, b3, w4, b4, w5, b5, **unused):
    out, _ = run(x, lens, emb, w3, b3, w4, b4, w5, b5)
    return out
